# revision 12
# baseline (speedup 1.0000x reference)
"""Trainium2 Bass kernel for nn_DfNet (DeepFilterNet-style dense_cnn).

Sharding: data-parallel over batch (B=16 -> 2 per core x 8 cores); each core
runs the full network on its 2 samples. Host pre-packs weights (conv blocks
folded into dense (channel,freq) band matrices) and input layouts; host
post-reshapes outputs.

Device phases per core:
  P2  encoder convs as matmuls over (c,f), N=(b,t) slices
  P3  cemb + e3 -> emb_pre
  P4  5 GRU layers chunk-pipelined (wavefront over 125-step chunks);
      recurrent matmuls weight-stationary bf16; gates on DVE/ACT
  P5  decoder convs, erb mask, deep filtering, output assembly
"""
import os
import sys

for _p in ("/opt/trn_rl_repo",):
    if _p not in sys.path and os.path.isdir(_p):
        sys.path.insert(0, _p)

import numpy as np
import ml_dtypes

import concourse.bacc as bacc
import concourse.bass as bass
import concourse.mybir as mybir
from concourse.tile import TileContext

f32 = mybir.dt.float32
bf16 = mybir.dt.bfloat16
ALU = mybir.AluOpType
AF = mybir.ActivationFunctionType
bfnp = ml_dtypes.bfloat16

B = 16
BL = 2
NC = 8
T = int(os.environ.get("KERNEL_T", "1000"))
TCH = 125
NCH = T // TCH
F = 481
CH = 16
TP = T + 8
DEBUG = bool(int(os.environ.get("KERNEL_DEBUG", "0")))

GNAMES = ["g0", "g1", "g2", "g3", "g4"]
GDEPTH = [0, 1, 1, 2, 3]
GPRED = {1: 0, 2: 0, 3: 2, 4: 3}
WIH_KC = [1, 2, 2, 2, 2]


# ======================= host: weight packing =======================
def build_conv_A(p, cin, cout, Fin, kt, kf, stride_f=1, groups=1, transpose=False,
                 in_fmajor=False, out_order=None):
    dw = np.asarray(p["dw"], np.float32)
    g_ = np.asarray(p["g"], np.float32)
    b_ = np.asarray(p["b"], np.float32)
    pw = np.asarray(p["pw"], np.float32)[:, :, 0, 0] if "pw" in p else None
    Fout = Fin * 2 if transpose else (Fin + 2 * ((kf - 1) // 2) - kf) // stride_f + 1
    cpg_in, cpg_out = cin // groups, cout // groups
    K, M = cin * Fin, cout * Fout

    def iidx(ci, fi):
        return (fi * cin + ci) if in_fmajor else (ci * Fin + fi)

    A = [np.zeros((M, K), np.float32) for _ in range(kt)]
    for co in range(cout):
        grp = co // cpg_out
        for fo in range(Fout):
            m = co * Fout + fo
            for dt in range(kt):
                for df in range(kf):
                    for cil in range(cpg_in):
                        ci = grp * cpg_in + cil
                        if transpose:
                            j = fo + df - 1
                            if j < 0 or j % 2 != 0:
                                continue
                            fi = j // 2
                            if fi >= Fin:
                                continue
                        else:
                            fi = stride_f * fo + df - (kf - 1) // 2
                            if fi < 0 or fi >= Fin:
                                continue
                        A[dt][m, iidx(ci, fi)] += dw[co, cil, dt, df]
    if pw is not None:
        P_ = np.zeros((M, M), np.float32)
        for co in range(cout):
            for cm in range(cout):
                for fo in range(Fout):
                    P_[co * Fout + fo, cm * Fout + fo] = pw[co, cm]
        A = [P_ @ a for a in A]
    A = [np.repeat(g_, Fout)[:, None] * a for a in A]
    bias = np.repeat(b_, Fout)
    if out_order == "fmajor":
        gather = np.array([c * Fout + f for f in range(Fout) for c in range(cout)])
        A = [a[gather] for a in A]
        bias = bias[gather]
    return A, bias, Fout


def pack_lhsT(A, kchunks=None):
    """A [M,K] -> [128, nKc*nMc*128] f32; block (kc,mc) at (kc*nMc+mc)*128."""
    M, K = A.shape
    if kchunks is None:
        nKc = (K + 127) // 128
        kchunks = [min(128, K - i * 128) for i in range(nKc)]
    nKc = len(kchunks)
    nMc = (M + 127) // 128
    out = np.zeros((128, nKc * nMc * 128), np.float32)
    k0 = 0
    for kc, kk in enumerate(kchunks):
        for mc in range(nMc):
            mm = min(128, M - mc * 128)
            out[:kk, (kc * nMc + mc) * 128 : (kc * nMc + mc) * 128 + mm] = (
                A[mc * 128 : mc * 128 + mm, k0 : k0 + kk].T
            )
        k0 += kk
    return out


def pack_col(v):
    nb = (len(v) + 127) // 128
    out = np.zeros((128, nb), np.float32)
    for i in range(nb):
        seg = v[i * 128 : (i + 1) * 128]
        out[: len(seg), i] = seg
    return out


def prep_weights(params):
    pe, pm, pd = params["enc"], params["erb_dec"], params["df_dec"]
    W = {}

    def add(name, A_list, bias, kchunks=None):
        W[name + "_A"] = np.concatenate(
            [pack_lhsT(a, kchunks) for a in A_list], axis=1
        ).astype(bfnp)
        W[name + "_b"] = pack_col(bias)

    A0, b0, _ = build_conv_A(pe["erb_conv0"], 1, CH, 32, 3, 3)
    add("e0", A0, b0, kchunks=[32])
    A1, b1, _ = build_conv_A(pe["erb_conv1"], CH, CH, 32, 1, 3, 2, CH)
    add("e1", A1, b1)
    A2, b2, _ = build_conv_A(pe["erb_conv2"], CH, CH, 16, 1, 3, 2, CH)
    add("e2", A2, b2)
    A3, b3, _ = build_conv_A(pe["erb_conv3"], CH, CH, 8, 1, 3, 1, CH,
                             out_order="fmajor")
    add("e3", A3, b3)
    Ac0, bc0, _ = build_conv_A(pe["df_conv0"], 2, CH, 96, 3, 3, 1, 2)
    add("c0", Ac0, bc0, kchunks=[96, 96])
    Ac1, bc1, _ = build_conv_A(pe["df_conv1"], CH, CH, 96, 1, 3, 2, CH)
    add("c1", Ac1, bc1)

    Wfc = np.asarray(pe["df_fc_emb"]["W"], np.float32)
    Wfc_dev = np.zeros_like(Wfc)
    for c in range(16):
        for f in range(48):
            Wfc_dev[:, c * 48 + f] = Wfc[:, f * 16 + c]
    add("cemb", [Wfc_dev], np.asarray(pe["df_fc_emb"]["b"], np.float32))

    for nm, g in zip(GNAMES, [pe["emb_gru"][0], pm["emb_gru"][0]] + list(pd["df_gru"])):
        Whh = np.asarray(g["Whh"], np.float32)
        Wih = np.asarray(g["Wih"], np.float32)
        bih = np.asarray(g["bih"], np.float32)
        bhh = np.asarray(g["bhh"], np.float32)
        W[nm + "_whh"] = pack_lhsT(Whh).astype(bfnp)
        W[nm + "_wih"] = pack_lhsT(Wih).astype(bfnp)
        gib = bih + np.concatenate([bhh[:512], np.zeros(256, np.float32)])
        W[nm + "_gib"] = np.stack([gib[i * 128 : (i + 1) * 128] for i in range(6)], 1)
        W[nm + "_bhhn"] = np.stack([bhh[512:640], bhh[640:768]], axis=1)

    W["lsnr_W"] = pack_lhsT(np.asarray(pe["lsnr_fc"]["W"], np.float32)).astype(bfnp)
    W["lsnr_b"] = np.asarray(pe["lsnr_fc"]["b"], np.float32).reshape(1, 1)
    W["alpha_W"] = pack_lhsT(np.asarray(pd["df_fc_a"]["W"], np.float32)).astype(bfnp)
    W["alpha_b"] = np.asarray(pd["df_fc_a"]["b"], np.float32).reshape(1, 1)

    add("demb", [np.asarray(pm["fc_emb"]["W"], np.float32)],
        np.asarray(pm["fc_emb"]["b"], np.float32))

    def dwise(pp, fmajor, Fv):
        dwv = np.asarray(pp["dw"], np.float32)[:, 0, 0, 0]
        g_ = np.asarray(pp["g"], np.float32)
        b_ = np.asarray(pp["b"], np.float32)
        if fmajor:
            return pack_col(np.tile(dwv * g_, Fv)), pack_col(np.tile(b_, Fv))
        return pack_col(np.repeat(dwv * g_, Fv)), pack_col(np.repeat(b_, Fv))

    W["c3p_s"], W["c3p_b"] = dwise(pm["conv3p"], True, 8)
    W["c2p_s"], W["c2p_b"] = dwise(pm["conv2p"], False, 8)
    W["c1p_s"], W["c1p_b"] = dwise(pm["conv1p"], False, 16)
    W["c0p_s"], W["c0p_b"] = dwise(pm["conv0p"], False, 32)

    At3, bt3, _ = build_conv_A(pm["convt3"], CH, CH, 8, 1, 3, 1, CH, in_fmajor=True)
    add("xt3", At3, bt3)
    At2, bt2, _ = build_conv_A(pm["convt2"], CH, CH, 8, 1, 3, 1, CH, transpose=True)
    add("xt2", At2, bt2)
    At1, bt1, _ = build_conv_A(pm["convt1"], CH, CH, 16, 1, 3, 1, CH, transpose=True)
    add("xt1", At1, bt1)
    A0o, b0o, _ = build_conv_A(pm["conv0_out"], CH, 1, 32, 1, 3, 1, 1)
    add("mout", A0o, b0o)

    Wdf = np.asarray(pd["df_out"]["W"], np.float32)
    bdf = np.asarray(pd["df_out"]["b"], np.float32)
    Wdf_pl = np.zeros((10 * 128, 256), np.float32)
    bdf_pl = np.zeros(10 * 128, np.float32)
    for och in range(10):
        Wdf_pl[och * 128 : och * 128 + 96] = Wdf[och::10][:96]
        bdf_pl[och * 128 : och * 128 + 96] = bdf[och::10][:96]
    add("dfo", [Wdf_pl], bdf_pl)

    Acp, bcp, _ = build_conv_A(pd["df_convp"], CH, 10, 96, 1, 1, 1, 2)
    Acp_pl = np.zeros((10 * 128, Acp[0].shape[1]), np.float32)
    bcp_pl = np.zeros(10 * 128, np.float32)
    for och in range(10):
        Acp_pl[och * 128 : och * 128 + 96] = Acp[0][och * 96 : (och + 1) * 96]
        bcp_pl[och * 128 : och * 128 + 96] = bcp[och * 96 : (och + 1) * 96]
    add("cvp", [Acp_pl], bcp_pl)
    return W


def prep_inputs(spec, feat_erb, feat_spec, erb_fb):
    spec = np.asarray(spec, np.float32)
    feat_erb = np.asarray(feat_erb, np.float32)
    feat_spec = np.asarray(feat_spec, np.float32)
    erb_fb = np.asarray(erb_fb, np.float32)
    erbfb_pk = pack_lhsT(erb_fb.T).astype(bfnp)
    maps = []
    for c in range(NC):
        sl = slice(c * BL, (c + 1) * BL)
        m = {}
        a = np.zeros((32, BL, TP), np.float32)
        a[:, :, 8:] = feat_erb[sl, 0, :T].transpose(2, 0, 1)
        m["ferb"] = a.reshape(32, -1).astype(bfnp)
        a = np.zeros((2, 96, BL, TP), np.float32)
        a[:, :, :, 8:] = feat_spec[sl, 0, :T].transpose(3, 2, 0, 1)
        m["fs"] = a.reshape(192, -1).astype(bfnp)
        sp = spec[sl, 0, :T]
        spT = sp.transpose(2, 0, 1, 3).reshape(481, -1)
        spf = np.zeros((4, 128, BL * T * 2), np.float32)
        for mc in range(4):
            ff = min(128, 481 - mc * 128)
            spf[mc, :ff] = spT[mc * 128 : mc * 128 + ff]
        m["spf"] = spf.astype(bfnp)
        a = np.zeros((96, BL, TP, 2), np.float32)
        a[:, :, 8:] = sp[:, :, :96].transpose(2, 0, 1, 3)
        m["sdf"] = a.reshape(96, -1).astype(bfnp)
        m["erbfb"] = erbfb_pk
        maps.append(m)
    return maps


# ======================= device program =======================
class Ctx:
    def __init__(self, nc, tc, d):
        self.nc, self.tc, self.d = nc, tc, d
        self.WT = {}

    def loadw(self, pool, name, dt=bf16):
        shape = list(self.d[name].shape)
        t = pool.tile(shape, dt, name="w_" + name, tag="w_" + name)
        self.nc.sync.dma_start(out=t[:], in_=self.d[name][:])
        self.WT[name] = t
        return t


def conv_layer(p, pspool, name, x_tiles, out_name, nMc, ktaps, act, kchunks,
               pool, out_dt=bf16, xoff=8, xstride=None, Mtail=128,
               trange=None, out_tiles=None, tag=None):
    """x_tiles[kc]: [kk, BL*xstride]. Out tiles [mw, BL*(t1-t0)]."""
    nc = p.nc
    A = p.WT[name + "_A"]
    bias = p.WT[name + "_b"]
    nKc = len(kchunks)
    xs = xstride if xstride is not None else TP
    lo, hi = trange if trange else (0, T)
    oT = hi - lo
    if out_tiles is None:
        out_tiles = []
        for mc in range(nMc):
            mw = Mtail if mc == nMc - 1 else 128
            out_tiles.append(pool.tile(
                [mw, BL * oT], out_dt, name=f"{out_name}{mc}",
                tag=(tag or f"{out_name}{mc}")))
    for mc in range(nMc):
        mw = Mtail if mc == nMc - 1 else 128
        for b in range(BL):
            for t0 in range(lo, hi, 500):
                ln = min(500, hi - t0)
                ps = pspool.tile([128, 500], f32, tag="convps",
                                 name=f"ps_{out_name}_{mc}_{b}_{t0}")
                nmm = 0
                for dt in range(ktaps):
                    for kc in range(nKc):
                        kk = kchunks[kc]
                        col0 = b * xs + xoff + t0 + (dt - (ktaps - 1))
                        blk = (dt * nKc * nMc + kc * nMc + mc) * 128
                        nc.tensor.matmul(
                            ps[:mw, :ln],
                            A[:kk, blk : blk + mw],
                            x_tiles[kc][:kk, col0 : col0 + ln],
                            start=(nmm == 0),
                            stop=(nmm == ktaps * nKc - 1),
                        )
                        nmm += 1
                nc.scalar.activation(
                    out_tiles[mc][:mw, b * oT + (t0 - lo) : b * oT + (t0 - lo) + ln],
                    ps[:mw, :ln],
                    act if act is not None else AF.Identity,
                    bias=bias[:mw, mc : mc + 1],
                )
    return out_tiles


def hs_col(kc, b, t):
    return kc * (BL * (T + 1)) + b * (T + 1) + t


def build_body(p, tc, dram_c0p, dbg):
    nc, d = p.nc, p.d
    iopool = tc.alloc_tile_pool(name="iopool", bufs=1)
    keep = tc.alloc_tile_pool(name="keep", bufs=1)       # emb_pre, hs4
    epool = tc.alloc_tile_pool(name="epool", bufs=1)     # e0..e3
    pspool = tc.alloc_tile_pool(name="pspool", bufs=3, space="PSUM")

    # ---------- inputs ----------
    ferb = iopool.tile([32, BL * TP], bf16, name="ferb_sb")
    nc.sync.dma_start(out=ferb[:], in_=d["ferb"][:])
    fs0 = iopool.tile([96, BL * TP], bf16, name="fs0_sb")
    fs1 = iopool.tile([96, BL * TP], bf16, name="fs1_sb")
    nc.sync.dma_start(out=fs0[:], in_=d["fs"][0:96, :])
    nc.sync.dma_start(out=fs1[:], in_=d["fs"][96:192, :])

    # ---------- P2 ----------
    p2w = tc.alloc_tile_pool(name="p2w", bufs=1)
    for nm in ["e0", "e1", "e2", "e3", "c0", "c1", "cemb", "cvp"]:
        p.loadw(p2w, nm + "_A")
        p.loadw(p2w, nm + "_b", f32)

    e0 = conv_layer(p, pspool, "e0", [ferb], "e0", 4, 3, AF.Relu, [32], epool)
    e1 = conv_layer(p, pspool, "e1", e0, "e1", 2, 1, AF.Relu, [128] * 4, epool,
                    xoff=0, xstride=T)
    e2 = conv_layer(p, pspool, "e2", e1, "e2", 1, 1, AF.Relu, [128] * 2, epool,
                    xoff=0, xstride=T)
    e3 = conv_layer(p, pspool, "e3", e2, "e3", 1, 1, AF.Relu, [128], epool,
                    xoff=0, xstride=T)

    emb_pre = keep.tile([128, BL * T], bf16, name="emb_pre")
    cpool = tc.alloc_tile_pool(name="cpool", bufs=1)
    HT = min(500, T)  # c-branch processed in T-halves to bound SBUF
    for lo in range(0, T, HT):
        hi = lo + HT
        c0 = conv_layer(p, pspool, "c0", [fs0, fs1], "c0", 12, 3, AF.Relu,
                        [96, 96], cpool, trange=(lo, hi))
        c1 = conv_layer(p, pspool, "c1", c0, "c1", 6, 1, AF.Relu, [128] * 12,
                        cpool, xoff=0, xstride=HT, trange=(0, HT))
        c0p = conv_layer(p, pspool, "cvp", c0, "c0p", 10, 1, AF.Relu, [128] * 12,
                         cpool, xoff=0, xstride=HT, trange=(0, HT))
        for och in range(10):
            cp3 = dram_c0p[:].rearrange("(o p) (b t) -> o p b t", o=10, b=BL)
            for b in range(BL):
                nc.sync.dma_start(
                    out=cp3[och, :, b, lo:hi],
                    in_=c0p[och][:, b * HT : (b + 1) * HT])
        cemb = conv_layer(p, pspool, "cemb", c1, "cemb", 1, 1, None, [128] * 6,
                          cpool, xoff=0, xstride=HT, trange=(0, HT))
        for b in range(BL):
            nc.vector.tensor_tensor(
                out=emb_pre[:, b * T + lo : b * T + hi],
                in0=cemb[0][:, b * HT : (b + 1) * HT],
                in1=e3[0][:, b * T + lo : b * T + hi], op=ALU.add)
    cpool.release()
    p2w.release()
    if DEBUG:
        nc.sync.dma_start(out=dbg["dbg_embpre"][:], in_=emb_pre[:])

    # ---------- P4 ----------
    p4w = tc.alloc_tile_pool(name="p4w", bufs=1)
    for g in GNAMES:
        p.loadw(p4w, g + "_whh")
        p.loadw(p4w, g + "_wih")
        p.loadw(p4w, g + "_gib", f32)
        p.loadw(p4w, g + "_bhhn", f32)
    for nm in ["lsnr_W", "demb_A"]:
        p.loadw(p4w, nm)
    for nm in ["lsnr_b", "demb_b"]:
        p.loadw(p4w, nm, f32)
    hsB = tc.alloc_tile_pool(name="hsB", bufs=1)  # hs0..hs3 (die after P4)
    hstreams = []
    for i in range(5):
        hp = keep if i == 4 else hsB
        hs = hp.tile([128, 2 * BL * (T + 1)], bf16, name=f"hs{i}")
        nc.vector.memset(hs[:, :], 0.0)
        hstreams.append(hs)
    gi_tiles = [p4w.tile([128, 6 * BL * TCH], f32, name=f"gi{i}", tag=f"gi{i}", bufs=2)
                for i in range(5)]
    spool4 = tc.alloc_tile_pool(name="spool4", bufs=4)
    gps = tc.alloc_tile_pool(name="gps", bufs=1, space="PSUM")

    def build_gi(l, q):
        g = GNAMES[l]
        wih, gib = p.WT[g + "_wih"], p.WT[g + "_gib"]
        nkc = WIH_KC[l]
        for mc in range(6):
            for b in range(BL):
                ps = pspool.tile([128, TCH], f32, tag="convps",
                                 name=f"gips_{l}_{q}_{mc}_{b}")
                for kc in range(nkc):
                    if l == 0:
                        src = emb_pre[:, b * T + q * TCH : b * T + (q + 1) * TCH]
                    else:
                        hsrc = hstreams[GPRED[l]]
                        cc = hs_col(kc, b, q * TCH + 1)
                        src = hsrc[:, cc : cc + TCH]
                    nc.tensor.matmul(
                        ps[:, :TCH],
                        wih[:, (kc * 6 + mc) * 128 : (kc * 6 + mc) * 128 + 128],
                        src,
                        start=(kc == 0), stop=(kc == nkc - 1))
                nc.scalar.activation(
                    gi_tiles[l][:, (mc * BL + b) * TCH : (mc * BL + b + 1) * TCH],
                    ps[:, :TCH], AF.Identity, bias=gib[:, mc : mc + 1])

    def scan_chunk(l, q):
        g = GNAMES[l]
        whh, bhhn = p.WT[g + "_whh"], p.WT[g + "_bhhn"]
        hs = hstreams[l]
        hs4 = hs[:].rearrange("p (kc b t) -> p kc b t", kc=2, b=BL)
        gi4 = gi_tiles[l][:].rearrange("p (mc b t) -> p mc b t", mc=6, b=BL)
        for tt in range(TCH):
            t = q * TCH + tt
            ps = gps.tile([128, 6 * BL], f32, tag=f"sps{l}", name=f"sps_{l}_{t}")
            ps3 = ps[:].rearrange("p (mc b) -> p mc b", mc=6)
            for mc in range(6):
                for kc in range(2):
                    nc.tensor.matmul(
                        ps3[:, mc],
                        whh[:, (kc * 6 + mc) * 128 : (kc * 6 + mc) * 128 + 128],
                        hs4[:, kc, :, t],
                        start=(kc == 0), stop=(kc == 1))

            def sc(nm, w, dt):
                return spool4.tile([128, w], dt, tag=f"{nm}{l}", name=f"{nm}_{l}_{t}")

            rzpre = sc("rzpre", 4 * BL, f32)
            nc.vector.tensor_tensor(
                out=rzpre[:].rearrange("p (mc b) -> p mc b", mc=4),
                in0=ps3[:, 0:4], in1=gi4[:, 0:4, :, tt], op=ALU.add)
            rz = sc("rz", 4 * BL, bf16)
            nc.scalar.activation(rz[:], rzpre[:], AF.Sigmoid)
            rz3 = rz[:].rearrange("p (kc b) -> p kc b", kc=4)
            npre = sc("npre", 2 * BL, f32)
            npre3 = npre[:].rearrange("p (kc b) -> p kc b", kc=2)
            for kc in range(2):
                nc.vector.scalar_tensor_tensor(
                    out=npre3[:, kc], in0=ps3[:, 4 + kc],
                    scalar=bhhn[:, kc : kc + 1], in1=rz3[:, kc],
                    op0=ALU.add, op1=ALU.mult)
            npre2 = sc("npre2", 2 * BL, f32)
            nc.vector.tensor_tensor(
                out=npre2[:].rearrange("p (kc b) -> p kc b", kc=2),
                in0=npre3[:, :], in1=gi4[:, 4:6, :, tt], op=ALU.add)
            n_ = sc("nn", 2 * BL, bf16)
            nc.scalar.activation(n_[:], npre2[:], AF.Tanh)
            n3 = n_[:].rearrange("p (kc b) -> p kc b", kc=2)
            d_ = sc("dd", 2 * BL, bf16)
            nc.vector.tensor_tensor(
                out=d_[:].rearrange("p (kc b) -> p kc b", kc=2),
                in0=hs4[:, :, :, t], in1=n3, op=ALU.subtract)
            e_ = sc("ee", 2 * BL, bf16)
            nc.vector.tensor_tensor(out=e_[:], in0=rz3[:, 2:4], in1=d_[:],
                                    op=ALU.mult)
            nc.vector.tensor_tensor(
                out=hs4[:, :, :, t + 1], in0=n3,
                in1=e_[:].rearrange("p (kc b) -> p kc b", kc=2), op=ALU.add)

    for slot in range(NCH + 3):
        for l in range(5):
            q = slot - GDEPTH[l]
            if 0 <= q < NCH:
                build_gi(l, q)
                scan_chunk(l, q)
    spool4.release()
    gps.release()
    p4w.release()
    if DEBUG:
        for i in range(5):
            nc.sync.dma_start(out=dbg[f"dbg_hs{i}"][:], in_=hstreams[i][:])

    # ---------- P5 ----------
    p5w = tc.alloc_tile_pool(name="p5w", bufs=1)
    for nm in ["demb", "xt3", "xt2", "xt1", "mout", "dfo"]:
        p.loadw(p5w, nm + "_A")
        p.loadw(p5w, nm + "_b", f32)
    for nm in ["c3p_s", "c3p_b", "c2p_s", "c2p_b", "c1p_s", "c1p_b",
               "c0p_s", "c0p_b"]:
        p.loadw(p5w, nm, f32)
    p.loadw(p5w, "alpha_W")
    p.loadw(p5w, "alpha_b", f32)
    erbfb = p.loadw(p5w, "erbfb")
    xb = tc.alloc_tile_pool(name="xb", bufs=14)   # recycled [<=128, BL*T] bf16
    sc5 = tc.alloc_tile_pool(name="sc5", bufs=2)  # small scratch
    dfp = tc.alloc_tile_pool(name="dfp", bufs=1)  # DF accumulators etc.

    def xtile(nm, rows=128, dt=bf16):
        return xb.tile([rows, BL * T], dt, tag="xbuf", name=nm)

    def head(wname, bname, src_hs, out_nm, func):
        out = sc5.tile([1, BL * T], f32, tag=out_nm, name=out_nm)
        for b in range(BL):
            for t0 in range(0, T, 500):
                ln = min(500, T - t0)
                ps = pspool.tile([1, 500], f32, tag="convps",
                                 name=f"hps_{out_nm}_{b}_{t0}")
                for kc in range(2):
                    cc = hs_col(kc, b, t0 + 1)
                    nc.tensor.matmul(
                        ps[:, :ln], p.WT[wname][:, kc * 128 : kc * 128 + 1],
                        src_hs[:, cc : cc + ln],
                        start=(kc == 0), stop=(kc == 1))
                nc.scalar.activation(out[:, b * T + t0 : b * T + t0 + ln],
                                     ps[:, :ln], func,
                                     bias=p.WT[bname][0:1, 0:1])
        return out

    lsnr_sig = head("lsnr_W", "lsnr_b", hstreams[0], "lsnr_sig", AF.Sigmoid)
    lsnr_o = sc5.tile([1, BL * T], f32, tag="lsnr_o", name="lsnr_o")
    nc.scalar.activation(lsnr_o[:], lsnr_sig[:], AF.Copy, bias=-15.0, scale=50.0)
    nc.sync.dma_start(out=d["lsnr_out"][:], in_=lsnr_o[:])

    alpha_sig = head("alpha_W", "alpha_b", hstreams[4], "alpha_sig", AF.Sigmoid)
    nc.sync.dma_start(out=d["alpha_out"][:], in_=alpha_sig[:])
    alpha_bf = sc5.tile([1, BL * T], bf16, tag="alpha_bf", name="alpha_bf")
    nc.scalar.copy(alpha_bf[:], alpha_sig[:])

    # d (erb decoder fc) via conv_layer on h stream slices
    hs1v = [hstreams[1][:, hs_col(kc, 0, 0) : hs_col(kc, 0, 0) + BL * (T + 1)]
            for kc in range(2)]
    dt_ = conv_layer(p, pspool, "demb", hs1v, "dtile", 1, 1, AF.Relu, [128] * 2,
                     xb, xoff=1, xstride=T + 1, tag="xbuf")[0]

    def pconv_add(e_tiles, sname, bname, addend_tiles, out_nm):
        outs = []
        for i, et in enumerate(e_tiles):
            mw = et.shape[0]
            tmp = xtile(f"{out_nm}tmp{i}", mw)
            nc.scalar.activation(tmp[:], et[:], AF.Relu,
                                 bias=p.WT[bname][:mw, i : i + 1],
                                 scale=p.WT[sname][:mw, i : i + 1])
            o = xtile(f"{out_nm}{i}", mw)
            nc.vector.tensor_tensor(out=o[:], in0=tmp[:], in1=addend_tiles[i][:],
                                    op=ALU.add)
            outs.append(o)
        return outs

    x3in = pconv_add(e3, "c3p_s", "c3p_b", [dt_], "x3in")
    x3 = conv_layer(p, pspool, "xt3", x3in, "x3", 1, 1, AF.Relu, [128], xb,
                    xoff=0, xstride=T, tag="xbuf")
    x2in = pconv_add(e2, "c2p_s", "c2p_b", x3, "x2in")
    x2 = conv_layer(p, pspool, "xt2", x2in, "x2", 2, 1, AF.Relu, [128], xb,
                    xoff=0, xstride=T, tag="xbuf")
    x1in = pconv_add(e1, "c1p_s", "c1p_b", x2, "x1in")
    x1 = conv_layer(p, pspool, "xt1", x1in, "x1", 4, 1, AF.Relu, [128] * 2, xb,
                    xoff=0, xstride=T, tag="xbuf")
    x0in = pconv_add(e0, "c0p_s", "c0p_b", x1, "x0in")
    m_sb = conv_layer(p, pspool, "mout", x0in, "m_sb", 1, 1, AF.Sigmoid,
                      [128] * 4, dfp, xoff=0, xstride=T, Mtail=32)[0]
    m_f32 = sc5.tile([32, BL * T], f32, tag="m_f32", name="m_f32")
    nc.scalar.copy(m_f32[:], m_sb[:])
    nc.sync.dma_start(out=d["m_out"][:], in_=m_f32[:])

    mask = []
    for mc in range(4):
        mk = dfp.tile([128, BL * T], bf16, tag=f"mask{mc}", name=f"mask{mc}")
        for b in range(BL):
            for t0 in range(0, T, 500):
                ln = min(500, T - t0)
                ps = pspool.tile([128, 500], f32, tag="convps",
                                 name=f"ps_mask{mc}_{b}_{t0}")
                nc.tensor.matmul(ps[:, :ln],
                                 erbfb[0:32, mc * 128 : mc * 128 + 128],
                                 m_sb[0:32, b * T + t0 : b * T + t0 + ln],
                                 start=True, stop=True)
                nc.scalar.activation(mk[:, b * T + t0 : b * T + t0 + ln],
                                     ps[:, :ln], AF.Copy)
        mask.append(mk)
    xb.release()
    epool.release()
    hsA.release()

    ones_sb = iopool.tile([1, 128], bf16, name="ones_sb")
    nc.vector.memset(ones_sb[:], 1.0)
    alpha_bc = dfp.tile([128, BL * T], bf16, tag="alpha_bc", name="alpha_bc")
    for b in range(BL):
        for t0 in range(0, T, 500):
            ln = min(500, T - t0)
            ps = pspool.tile([128, 500], f32, tag="convps", name=f"abc_{b}_{t0}")
            nc.tensor.matmul(ps[:, :ln], ones_sb[:, 0:128],
                             alpha_bf[:, b * T + t0 : b * T + t0 + ln],
                             start=True, stop=True)
            nc.scalar.activation(alpha_bc[:, b * T + t0 : b * T + t0 + ln],
                                 ps[:, :ln], AF.Copy)

    # deep filtering
    sdf = iopool.tile([96, BL * TP * 2], bf16, name="sdf_sb")
    nc.sync.dma_start(out=sdf[:], in_=d["sdf"][:])
    sdf4 = sdf[:].rearrange("p (b t r) -> p b t r", b=BL, t=TP)
    accr = [dfp.tile([96, T], f32, tag=f"accr{b}", name=f"accr{b}")
            for b in range(BL)]
    acci = [dfp.tile([96, T], f32, tag=f"acci{b}", name=f"acci{b}")
            for b in range(BL)]

    for o in range(5):
        # coefs planes 2o (real), 2o+1 (imag): tanh(dfo) + c0p
        pl = []
        for j in range(2):
            och = 2 * o + j
            co = sc5.tile([128, BL * T], bf16, tag="coef_s", name=f"co_{och}")
            for b in range(BL):
                for t0 in range(0, T, 500):
                    ln = min(500, T - t0)
                    ps = pspool.tile([128, 500], f32, tag="convps",
                                     name=f"cps_{och}_{b}_{t0}")
                    for kc in range(2):
                        cc = hs_col(kc, b, t0 + 1)
                        nc.tensor.matmul(
                            ps[:, :ln],
                            p.WT["dfo_A"][:, (kc * 10 + och) * 128 :
                                          (kc * 10 + och) * 128 + 128],
                            hstreams[4][:, cc : cc + ln],
                            start=(kc == 0), stop=(kc == 1))
                    nc.scalar.activation(co[:, b * T + t0 : b * T + t0 + ln],
                                         ps[:, :ln], AF.Tanh,
                                         bias=p.WT["dfo_b"][:, och : och + 1])
            cp = sc5.tile([128, BL * T], bf16, tag="c0pl", name=f"cp_{och}")
            nc.sync.dma_start(out=cp[:], in_=dram_c0p[och * 128 : (och + 1) * 128, :])
            cof = sc5.tile([128, BL * T], bf16, tag="coefs", name=f"cofs_{och}")
            nc.vector.tensor_tensor(out=cof[:], in0=co[:], in1=cp[:], op=ALU.add)
            pl.append(cof)
        for b in range(BL):
            cr = pl[0][0:96, b * T : (b + 1) * T]
            ci = pl[1][0:96, b * T : (b + 1) * T]
            sr = sdf4[0:96, b, 4 + o : 4 + o + T, 0]
            si = sdf4[0:96, b, 4 + o : 4 + o + T, 1]
            t1 = sc5.tile([96, T], f32, tag="df_t1", name=f"t1_{b}_{o}")
            t2 = sc5.tile([96, T], f32, tag="df_t2", name=f"t2_{b}_{o}")
            nc.vector.tensor_tensor(out=t1[:], in0=cr, in1=sr, op=ALU.mult)
            nc.vector.tensor_tensor(out=t2[:], in0=ci, in1=si, op=ALU.mult)
            if o == 0:
                nc.vector.tensor_tensor(out=accr[b][:], in0=t1[:], in1=t2[:],
                                        op=ALU.subtract)
            else:
                t5 = sc5.tile([96, T], f32, tag="df_t5", name=f"t5_{b}_{o}")
                nc.vector.tensor_tensor(out=t5[:], in0=t1[:], in1=t2[:],
                                        op=ALU.subtract)
                nc.vector.tensor_tensor(out=accr[b][:], in0=accr[b][:],
                                        in1=t5[:], op=ALU.add)
            t3 = sc5.tile([96, T], f32, tag="df_t1", name=f"t3_{b}_{o}")
            t4 = sc5.tile([96, T], f32, tag="df_t2", name=f"t4_{b}_{o}")
            nc.vector.tensor_tensor(out=t3[:], in0=cr, in1=si, op=ALU.mult)
            nc.vector.tensor_tensor(out=t4[:], in0=ci, in1=sr, op=ALU.mult)
            if o == 0:
                nc.vector.tensor_tensor(out=acci[b][:], in0=t3[:], in1=t4[:],
                                        op=ALU.add)
            else:
                t6 = sc5.tile([96, T], f32, tag="df_t5", name=f"t6_{b}_{o}")
                nc.vector.tensor_tensor(out=t6[:], in0=t3[:], in1=t4[:],
                                        op=ALU.add)
                nc.vector.tensor_tensor(out=acci[b][:], in0=acci[b][:],
                                        in1=t6[:], op=ALU.add)

    # spec_m / low mix / outputs
    for mc in range(4):
        spt = sc5.tile([128, BL * T * 2], bf16, tag="spf_in", name=f"spf_{mc}", bufs=1)
        nc.sync.dma_start(out=spt[:], in_=d["spf"][mc, :, :])
        out_sb = sc5.tile([128, BL * T * 2], f32, tag="spec_o", name=f"spo_{mc}", bufs=1)
        spt3 = spt[:].rearrange("p (bt r) -> p bt r", r=2)
        out3 = out_sb[:].rearrange("p (bt r) -> p bt r", r=2)
        if mc == 0:
            spm = dfp.tile([96, BL * T * 2], f32, tag="spm_low", name="spm_low")
            spm3 = spm[:].rearrange("p (bt r) -> p bt r", r=2)
            for r in range(2):
                nc.vector.tensor_tensor(out=spm3[:, :, r], in0=spt3[0:96, :, r],
                                        in1=mask[0][0:96, :], op=ALU.mult)
            for b in range(BL):
                acc = [accr[b], acci[b]]
                for r in range(2):
                    spm_sl = spm3[:, b * T : (b + 1) * T, r]
                    d1 = sc5.tile([96, T], f32, tag="mix_d1", name=f"d1_{b}_{r}", bufs=1)
                    nc.vector.tensor_tensor(out=d1[:], in0=acc[r][:], in1=spm_sl,
                                            op=ALU.subtract)
                    d2 = sc5.tile([96, T], f32, tag="mix_d2", name=f"d2_{b}_{r}", bufs=1)
                    nc.vector.tensor_tensor(
                        out=d2[:], in0=d1[:],
                        in1=alpha_bc[0:96, b * T : (b + 1) * T], op=ALU.mult)
                    nc.vector.tensor_tensor(
                        out=out3[0:96, b * T : (b + 1) * T, r],
                        in0=spm_sl, in1=d2[:], op=ALU.add)
            for r in range(2):
                nc.vector.tensor_tensor(out=out3[96:128, :, r],
                                        in0=spt3[96:128, :, r],
                                        in1=mask[0][96:128, :], op=ALU.mult)
        else:
            for r in range(2):
                nc.vector.tensor_tensor(out=out3[:, :, r], in0=spt3[:, :, r],
                                        in1=mask[mc][:, :], op=ALU.mult)
        nc.sync.dma_start(out=d["spec_out"][mc, :, :], in_=out_sb[:])

    for pool in (sc5, dfp, p5w, pspool, keep, iopool):
        pool.release()


def build_program(wshapes):
    nc = bacc.Bacc()
    d = {}

    def din(name, shape, dt):
        d[name] = nc.declare_dram_parameter(name, list(shape), dt, isOutput=False)

    din("ferb", [32, BL * TP], bf16)
    din("fs", [192, BL * TP], bf16)
    din("spf", [4, 128, BL * T * 2], bf16)
    din("sdf", [96, BL * TP * 2], bf16)
    din("erbfb", [128, 4 * 128], bf16)
    for k, v in wshapes.items():
        din(k, v[0], bf16 if v[1] == "bf16" else f32)
    d["spec_out"] = nc.declare_dram_parameter("spec_out", [4, 128, BL * T * 2],
                                              f32, isOutput=True)
    d["m_out"] = nc.declare_dram_parameter("m_out", [32, BL * T], f32,
                                           isOutput=True)
    d["lsnr_out"] = nc.declare_dram_parameter("lsnr_out", [1, BL * T], f32,
                                              isOutput=True)
    d["alpha_out"] = nc.declare_dram_parameter("alpha_out", [1, BL * T], f32,
                                               isOutput=True)
    dbg = {}
    if DEBUG:
        dbg["dbg_embpre"] = nc.declare_dram_parameter("dbg_embpre",
                                                      [128, BL * T], bf16,
                                                      isOutput=True)
        for i in range(5):
            dbg[f"dbg_hs{i}"] = nc.declare_dram_parameter(
                f"dbg_hs{i}", [128, 2 * BL * (T + 1)], bf16, isOutput=True)
        d.update(dbg)
    dram_c0p = nc.dram_tensor("c0p_spill", [10 * 128, BL * T], bf16)
    with TileContext(nc) as tc:
        p = Ctx(nc, tc, d)
        build_body(p, tc, dram_c0p, dbg)
    nc.compile()
    return nc


_cache = {}


def _ensure(params):
    W = prep_weights(params)
    if "nc" not in _cache:
        wshapes = {k: (v.shape, "bf16" if v.dtype == bfnp else "f32")
                   for k, v in W.items()}
        _cache["nc"] = build_program(wshapes)
    return W


def kernel(spec, feat_erb, feat_spec, erb_fb, params):
    W = _ensure(params)
    nc = _cache["nc"]
    in_maps = prep_inputs(spec, feat_erb, feat_spec, erb_fb)
    for m in in_maps:
        m.update(W)
    if "runner" not in _cache:
        try:
            from tk import make_runner
            _cache["runner"] = make_runner(nc, NC)
        except Exception:
            _cache["runner"] = None
    if _cache.get("runner"):
        run, _ = _cache["runner"]
        results, times = run(in_maps, n_iters=1)
        _cache["last_times"] = times
    else:
        from concourse.bass_utils import run_bass_kernel_spmd
        results = run_bass_kernel_spmd(nc, in_maps, list(range(NC))).results
    _cache["results"] = results

    spec_out = np.zeros((B, T, F, 2), np.float32)
    m_out = np.zeros((B, 1, T, 32), np.float32)
    lsnr = np.zeros((B, T, 1), np.float32)
    alpha = np.zeros((B, T, 1), np.float32)
    for c in range(NC):
        r = results[c]
        so = r["spec_out"].reshape(4, 128, BL, T, 2)
        for mc in range(4):
            ff = min(128, 481 - mc * 128)
            spec_out[c * BL : (c + 1) * BL, :, mc * 128 : mc * 128 + ff] = (
                so[mc, :ff].transpose(1, 2, 0, 3))
        m_out[c * BL : (c + 1) * BL, 0] = (
            r["m_out"].reshape(32, BL, T).transpose(1, 2, 0))
        lsnr[c * BL : (c + 1) * BL, :, 0] = r["lsnr_out"].reshape(BL, T)
        alpha[c * BL : (c + 1) * BL, :, 0] = r["alpha_out"].reshape(BL, T)
    return spec_out, m_out, lsnr, alpha
